# revision 1
# baseline (speedup 1.0000x reference)
"""Trainium2 Bass kernel for nn_Kernel_Conv_83554293776951.

Reference computation (batch-as-groups dynamic conv):
    y      = relu(W1 @ x + b1)                       per (b, hw)
    weight = W2 @ y + b2                             [b, O*C, hw]
    out[b,o] = sum_{c,hw} x[b,c,hw] * weight[b,(o,c),hw]

Algebraic rewrite (8x fewer FLOPs): contract hw first into a per-sample
Gram tensor G[b,c,k] = sum_hw x[b,c,hw]*y[b,k,hw], then
    out[b,o] = sum_{c,k} G[b,c,k]*W2[(o,c),k] + sum_c b2[(o,c)]*xs[b,c]
with xs[b,c] = sum_hw x[b,c,hw].  The attention branch of the reference
is dead code and is skipped.

Sharding: the contraction dim c (256) is split across 8 cores (32 each).
Each core streams its W2 slice (8MB as bf16) and produces a partial
[32, 512] output; partials are summed on the host.  The kernel is
DMA-bound on the W2 stream (~24us/core floor).
"""

import sys

for _p in ("/opt/trn_rl_repo",):
    if _p not in sys.path:
        sys.path.insert(0, _p)

import numpy as np
import ml_dtypes

B = 32          # batch
C = 256         # in channels (contraction "c")
KD = 256        # hidden dim of y (contraction "k")
O = 512         # out channels
HW = 16         # spatial 4x4
NCORES = 8
CS = C // NCORES  # 32 c-channels per core
NCHUNK = CS * (KD // 128)  # 64 contraction chunks of 128 per core
W2TILES = 8     # chunks grouped 8 per DMA tile

_CACHE = {}


def _build_nc():
    import concourse.bass as bass  # noqa: F401
    from concourse import bacc
    import concourse.mybir as mybir
    import concourse.tile as tile

    f32 = mybir.dt.float32
    f32r = mybir.dt.float32r
    bf16 = mybir.dt.bfloat16

    nc = bacc.Bacc(None, target_bir_lowering=False)

    with tile.TileContext(nc) as tc:
        with tc.tile_pool(name="dram", bufs=1, space="DRAM") as dram:
            xT_d = dram.tile([C, B * HW], bf16, kind="ExternalInput", uniquify=False, name="xT")
            xbd_d = dram.tile([128, 4, 8 * CS], bf16, kind="ExternalInput", uniquify=False, name="xbd")
            w1t_d = dram.tile([C, KD], bf16, kind="ExternalInput", uniquify=False, name="w1t")
            b1t_d = dram.tile([128, KD], f32, kind="ExternalInput", uniquify=False, name="b1t")
            w2t_d = dram.tile([NCHUNK, 128, O], bf16, kind="ExternalInput", uniquify=False, name="w2t")
            b2f_d = dram.tile([C, O], bf16, kind="ExternalInput", uniquify=False, name="b2f")
            out_d = dram.tile([B, O], f32, kind="ExternalOutput", uniquify=False, name="out")

            from contextlib import ExitStack

            stack = ExitStack()
            consts = stack.enter_context(tc.tile_pool(name="consts", bufs=1))
            w2pool = stack.enter_context(tc.tile_pool(name="w2pool", bufs=1))
            psum_y = stack.enter_context(
                tc.tile_pool(name="psum_y", bufs=1, space="PSUM")
            )
            psum_g = stack.enter_context(
                tc.tile_pool(name="psum_g", bufs=1, space="PSUM")
            )
            psum_o = stack.enter_context(
                tc.tile_pool(name="psum_o", bufs=1, space="PSUM")
            )

            # ---- constant-ish loads (small) ----
            xT_sb = consts.tile([128, 2, B * HW], bf16)  # [c_part, cc, (b,hw)]
            nc.sync.dma_start(
                out=xT_sb[:], in_=xT_d.rearrange("(cc p) f -> p cc f", p=128)
            )
            w1t_sb = consts.tile([128, 2, KD], bf16)  # [c_part, cc, k]
            nc.sync.dma_start(
                out=w1t_sb[:], in_=w1t_d.rearrange("(cc p) k -> p cc k", p=128)
            )
            # Small loads split onto the ACT HWDGE ring so they are not
            # queued behind the 8MB W2 stream.
            b1t_sb = consts.tile([128, KD], f32)
            nc.scalar.dma_start(out=b1t_sb[:], in_=b1t_d[:, :])
            b2f_sb = consts.tile([128, 2, O], bf16)  # [c_part, cc, o]
            nc.scalar.dma_start(
                out=b2f_sb[:], in_=b2f_d.rearrange("(cc p) o -> p cc o", p=128)
            )

            # ---- block-diagonal X for the Gram matmuls (host-prepared) ----
            # Xbd[(b,hw)%128, g, (b%8)*32 + c~] = x[b, c0+c~, hw] for b in group g
            xbd_sb = consts.tile([128, 4, 8 * CS], bf16)
            nc.scalar.dma_start(out=xbd_sb[:], in_=xbd_d[:, :, :])

            # ---- W2 stream split across both HWDGE rings (sync + scalar).
            # Each ring is FIFO, so the small loads above complete before the
            # 8MB W2 flood can starve them of HBM bandwidth. ----
            w2_sb = []
            for j in range(W2TILES):
                t = w2pool.tile([128, NCHUNK // W2TILES, O], bf16, name=f"w2sb{j}")
                eng = nc.sync if j % 2 == 0 else nc.scalar
                eng.dma_start(
                    out=t[:],
                    in_=w2t_d[j * 8 : (j + 1) * 8].rearrange("c k o -> k c o"),
                )
                w2_sb.append(t)

            # ---- step 1: y = relu(W1 @ x + b1) in [(b,hw) part, k] layout ----
            yps = psum_y.tile([128, 4, KD], f32)  # [(b,hw)%128, g, k]
            for g in range(4):
                for cc in range(2):
                    nc.tensor.matmul(
                        yps[:, g, :],
                        lhsT=xT_sb[:, cc, g * 128 : (g + 1) * 128],
                        rhs=w1t_sb[:, cc, :],
                        start=(cc == 0),
                        stop=(cc == 1),
                    )
            y_sb = consts.tile([128, 4, KD], bf16)
            for g in range(4):
                nc.vector.tensor_tensor(
                    out=y_sb[:, g, :],
                    in0=yps[:, g, :],
                    in1=b1t_sb[:],
                    op=mybir.AluOpType.add,
                )
                nc.vector.tensor_scalar_max(y_sb[:, g, :], y_sb[:, g, :], 0.0)

            # ---- step 2: Gram  G[k, b*32+c~] = sum_hw y[(b,hw),k] x[b,c~,hw] ----
            gps = [psum_g.tile([128, B * CS], f32, name=f"gps{kh}") for kh in range(2)]
            for kh in range(2):
                for g in range(4):
                    nc.tensor.matmul(
                        gps[kh][:, g * 256 : (g + 1) * 256],
                        lhsT=y_sb[:, g, kh * 128 : (kh + 1) * 128],
                        rhs=xbd_sb[:, g, :],
                        start=True,
                        stop=True,
                    )
            g_sb = [
                consts.tile([128, B * CS], bf16, name=f"gsb{kh}") for kh in range(2)
            ]
            for kh in range(2):
                nc.vector.tensor_copy(out=g_sb[kh][:], in_=gps[kh][:])

            # ---- xs[c, b] = sum_hw x[b, c, hw]  (for the b2 bias term) ----
            xs32_sb = consts.tile([128, 2, B], f32)
            for cc in range(2):
                nc.vector.tensor_reduce(
                    out=xs32_sb[:, cc, :],
                    in_=xT_sb[:, cc, :].rearrange("p (b h) -> p b h", h=HW),
                    axis=mybir.AxisListType.X,
                    op=mybir.AluOpType.add,
                )
            xs_sb = consts.tile([128, 2, B], bf16)
            nc.vector.tensor_copy(out=xs_sb[:], in_=xs32_sb[:])

            # ---- step 3: out[b, o] = sum_chunks G^T W2 + xs^T B2 ----
            ops = psum_o.tile([B, O], f32)
            for ch in range(NCHUNK):
                ct, kh = ch // 2, ch % 2
                lhsT = g_sb[kh].rearrange("p (b c) -> p c b", c=CS)[:, ct, :]
                rhs = w2_sb[ch // 8][:, ch % 8, :]
                nc.tensor.matmul(ops[:], lhsT=lhsT, rhs=rhs, start=(ch == 0), stop=False)
            for cc in range(2):
                nc.tensor.matmul(
                    ops[:],
                    lhsT=xs_sb[:, cc, :],
                    rhs=b2f_sb[:, cc, :],
                    start=False,
                    stop=(cc == 1),
                )

            out_sb = consts.tile([B, O], f32)
            nc.vector.tensor_copy(out=out_sb[:], in_=ops[:])
            nc.sync.dma_start(out=out_d[:, :], in_=out_sb[:])

            stack.close()

    nc.compile()
    return nc


def _prep_in_maps(x, W1, b1, W2, b2):
    bf = ml_dtypes.bfloat16
    x = np.ascontiguousarray(np.asarray(x, dtype=np.float32)).reshape(B, C, HW)
    W1 = np.asarray(W1, dtype=np.float32)
    b1 = np.asarray(b1, dtype=np.float32)
    W2 = np.asarray(W2, dtype=np.float32)
    b2 = np.asarray(b2, dtype=np.float32)

    xT = np.ascontiguousarray(x.transpose(1, 0, 2).reshape(C, B * HW)).astype(bf)
    w1t = np.ascontiguousarray(W1.T).astype(bf)
    b1t = np.ascontiguousarray(np.broadcast_to(b1, (128, KD)))
    W2r = W2.reshape(O, C, KD)
    b2r = b2.reshape(O, C)

    in_maps = []
    for i in range(NCORES):
        c0 = i * CS
        # [c~, k, o] -> chunks of [128k, 512o], chunk = c~*2 + kh
        w2s = np.ascontiguousarray(
            W2r[:, c0 : c0 + CS, :].transpose(1, 2, 0)
        ).astype(bf).reshape(NCHUNK, 128, O)
        b2f = np.zeros((C, O), dtype=bf)
        b2f[c0 : c0 + CS, :] = b2r[:, c0 : c0 + CS].T.astype(bf)
        xbd = np.zeros((128, 4, 8 * CS), dtype=bf)
        for b in range(B):
            g, j = b // 8, b % 8
            xbd[16 * j : 16 * (j + 1), g, CS * j : CS * (j + 1)] = (
                x[b, c0 : c0 + CS, :].T.astype(bf)
            )
        in_maps.append(
            {
                "xT": xT,
                "xbd": xbd,
                "w1t": w1t,
                "b1t": b1t,
                "w2t": w2s,
                "b2f": b2f,
            }
        )
    return in_maps


def kernel(x, W1, b1, W2, b2, Wa=None, ba=None, **_unused):
    from concourse.bass_utils import run_bass_kernel_spmd

    if "nc" not in _CACHE:
        _CACHE["nc"] = _build_nc()
    nc = _CACHE["nc"]

    in_maps = _prep_in_maps(x, W1, b1, W2, b2)
    res = run_bass_kernel_spmd(nc, in_maps, core_ids=list(range(NCORES)))
    partials = [r["out"].astype(np.float64) for r in res.results]
    out = np.sum(partials, axis=0).astype(np.float32)
    return out.reshape(B, O, 1, 1)

